# revision 17
# baseline (speedup 1.0000x reference)
"""ChildSumTreeLSTMCell on 8 Trainium2 NeuronCores.

Strategy: sort edges by destination node on the host (index preprocessing as
part of sharding), partition nodes contiguously across the 8 cores so every
core's segment sums are fully local -- zero collectives.  On each core, edges
are packed into node tiles (<=128 nodes, <=512 edges each, 4 chunks of 128
edge slots).  Segment sums become matmuls against a 0/1 membership matrix M
built on-device from the destination indices; the forget-gate gather
f[dst] * c_src factorizes to f * segment_sum(c_src), removing the second
scatter entirely.  All matmuls run in bf16 (inputs stay f32 in HBM).

v2: packed DMA streams (hce = h|c|e in one 768KB transfer, sfdf merged,
single merged [h|c] output), DMA issue spread across sync/gpsimd queues,
one-shot M build via free-dim broadcast, merged PSUM evacuation.
"""

import sys

for _p in ("/opt/trn_rl_repo", "/root/.axon_site/_ro/trn_rl_repo"):
    if _p not in sys.path:
        sys.path.append(_p)

import numpy as np
import ml_dtypes

import concourse.bacc as bacc
import concourse.mybir as mybir
import concourse.tile as tile
from concourse.bass_utils import run_bass_kernel_spmd

F32 = mybir.dt.float32
BF16 = mybir.dt.bfloat16

E = 500_000
N = 125_000
H = 128
G = 64
NCORES = 8
NPC = N // NCORES          # nodes per core
CHUNK = 128                # edges per chunk
CPT = 4                    # chunks per tile
TILE_E = CHUNK * CPT       # edge slots per tile
bf16_np = ml_dtypes.bfloat16

TRACE = False              # set by test.py to capture an NTFF profile
LAST = {}                  # last run's BassKernelResults


def _install_axon_hook():
    import types, contextlib, ctypes

    def _make_hook(so_path="/opt/axon/libaxon_pjrt.so"):
        lib = ctypes.CDLL(so_path)
        if not hasattr(lib, "axon_start_nrt_profile"):
            return None
        lib.axon_start_nrt_profile.argtypes = [
            ctypes.POINTER(ctypes.c_int64), ctypes.c_size_t]
        lib.axon_start_nrt_profile.restype = ctypes.c_int64
        lib.axon_stop_nrt_profile.argtypes = [ctypes.c_char_p]
        lib.axon_stop_nrt_profile.restype = ctypes.c_int64

        @contextlib.contextmanager
        def hook(output_dir, device_ids):
            import jax
            jax.devices()
            if device_ids:
                ids = (ctypes.c_int64 * len(device_ids))(*device_ids)
                rc = lib.axon_start_nrt_profile(ids, len(device_ids))
            else:
                rc = lib.axon_start_nrt_profile(None, 0)
            if rc != 0:
                raise RuntimeError("axon_start_nrt_profile rc=%d" % rc)
            try:
                yield
            finally:
                n = lib.axon_stop_nrt_profile(str(output_dir).encode())
                print("profile: %d file(s) written to %s" % (n, output_dir),
                      file=sys.stderr)

        return hook

    hook = _make_hook()
    mod = types.ModuleType("antenv.axon_hooks")
    mod.get_axon_ntff_profile_hook = lambda: hook
    mod.set_axon_ntff_profile_hook = lambda h: None
    sys.modules["antenv.axon_hooks"] = mod


def build_graph(T):
    """Build the per-core Bass graph for T node tiles."""
    nc = bacc.Bacc()
    dp = nc.declare_dram_parameter
    hce_ext = dp("hce", [T // 2, 128, 6 * TILE_E], BF16, isOutput=False)
    sfdf_ext = dp("sfdf", [T // 2, G, 4 * TILE_E], BF16, isOutput=False)
    oh4_ext = dp("oh4", [4, T * TILE_E], BF16, isOutput=False)
    dstT_ext = dp("dstT", [128, T * CPT], BF16, isOutput=False)
    welT_ext = dp("welT", [G, 128], F32, isOutput=False)
    wa_ext = dp("wa", [G, 4], F32, isOutput=False)
    bel_ext = dp("belB", [4, 128], F32, isOutput=False)
    wg_ext = {}
    for x in "fiuo":
        wg_ext[x] = dp("w%sT" % x, [2 * H, 128], F32, isOutput=False)
    bias_ext = {}
    for x in "fiuo":
        bias_ext[x] = (dp("bW%s" % x, [H], F32, isOutput=False),
                       dp("b%s" % x, [H], F32, isOutput=False))
    out_ext = dp("outT", [128, T * 2 * H], BF16, isOutput=True)

    with tile.TileContext(nc) as tc:
        cst = tc.alloc_tile_pool(name="cst", bufs=1)
        pin = tc.alloc_tile_pool(name="pin", bufs=8)
        pcv = tc.alloc_tile_pool(name="pcv", bufs=3)
        pnd = tc.alloc_tile_pool(name="pnd", bufs=3)
        pacc = tc.alloc_tile_pool(name="pacc", bufs=2, space="PSUM")
        pmm = tc.alloc_tile_pool(name="pmm", bufs=2, space="PSUM")

        # -- setup: constants -----------------------------------------------
        welT_sb = cst.tile([G, 128], F32)
        nc.sync.dma_start(out=welT_sb[:], in_=welT_ext[:])
        wa_sb = cst.tile([G, 4], F32)
        nc.sync.dma_start(out=wa_sb[:], in_=wa_ext[:])
        belr = cst.tile([4, 128], F32)
        nc.sync.dma_start(out=belr[:], in_=bel_ext[:])
        t2p = pmm.tile([4, 128], F32, tag="mm")
        nc.tensor.matmul(out=t2p[:], lhsT=wa_sb[:], rhs=welT_sb[:],
                         start=True, stop=True)
        wel_b16 = cst.tile([G, 128], BF16)
        nc.vector.tensor_copy(out=wel_b16[:], in_=welT_sb[:])
        t4b = cst.tile([4, 128], BF16)
        nc.vector.tensor_tensor(out=t4b[:], in0=t2p[:],
                                in1=belr[:], op=mybir.AluOpType.add)
        wtcomb = cst.tile([G + 4, 128], BF16)
        nc.sync.dma_start(out=wtcomb[0:G, :], in_=wel_b16[:])
        nc.sync.dma_start(out=wtcomb[G:G + 4, :], in_=t4b[:])

        wg = {}
        for x in "fiuo":
            stg = cst.tile([128, 128], F32, tag="wstg_%s" % x)
            nc.sync.dma_start(out=stg[:], in_=wg_ext[x][0:128, :])
            wa_t = cst.tile([128, 128], BF16, tag="wg_%s_a" % x)
            nc.vector.tensor_copy(out=wa_t[:], in_=stg[:])
            stg2 = cst.tile([128, 128], F32, tag="wstg2_%s" % x)
            nc.sync.dma_start(out=stg2[:], in_=wg_ext[x][128:256, :])
            wb_t = cst.tile([128, 128], BF16, tag="wg_%s_b" % x)
            nc.vector.tensor_copy(out=wb_t[:], in_=stg2[:])
            wg[x] = (wa_t, wb_t)

        bias = {}
        for x in "fiuo":
            b1 = cst.tile([128, 1], F32, tag="b1_%s" % x)
            nc.sync.dma_start(out=b1[:], in_=bias_ext[x][0][:, None])
            b2 = cst.tile([128, 1], F32, tag="b2_%s" % x)
            nc.sync.dma_start(out=b2[:], in_=bias_ext[x][1][:, None])
            bs = cst.tile([128, 1], F32, tag="bs_%s" % x)
            nc.vector.tensor_tensor(out=bs[:], in0=b1[:], in1=b2[:],
                                    op=mybir.AluOpType.add)
            bias[x] = bs

        it32 = cst.tile([128, 2 * TILE_E], mybir.dt.int32)
        nc.gpsimd.iota(out=it32[:], pattern=[[0, 2 * CPT], [1, CHUNK]], base=0,
                       channel_multiplier=0)
        iotaF = cst.tile([128, 2 * TILE_E], BF16)
        nc.vector.tensor_copy(out=iotaF[:], in_=it32[:])

        dstT = cst.tile([128, T * CPT], BF16)
        nc.sync.dma_start(out=dstT[:], in_=dstT_ext[:])

        # -- main loop: groups of 2 node tiles ------------------------------
        # hce cols (bf16): [h(t0) h(t1) | c(t0) c(t1) | e(t0) e(t1)]
        # sfdf cols (bf16): [sf(t0) sf(t1) | df(t0) df(t1)]
        # hc cols (f32):  [h(t0) h(t1) | c(t0) c(t1)] (host unmaps)
        assert T % 2 == 0
        AF = mybir.ActivationFunctionType
        TE2 = 2 * TILE_E
        for g in range(T // 2):
            hce = pin.tile([128, 3 * TE2], BF16, tag="hce")
            nc.sync.dma_start(out=hce[:], in_=hce_ext[g])
            sfdf = pin.tile([G, 2 * TE2], BF16, tag="sfdf")
            nc.scalar.dma_start(out=sfdf[:], in_=sfdf_ext[g])
            B68 = pcv.tile([G + 4, TE2], BF16, tag="B68")
            nc.sync.dma_start(
                out=B68[G:G + 4, :],
                in_=oh4_ext[:, g * TE2:(g + 1) * TE2])
            nc.vector.tensor_tensor(
                out=B68[0:G, :], in0=sfdf[:, 0:TE2], in1=sfdf[:, TE2:2 * TE2],
                op=mybir.AluOpType.mult)
            dstv = pcv.tile([128, TE2], BF16, tag="dstv")
            nc.gpsimd.tensor_copy(
                out=dstv[:].rearrange("p (c j) -> p c j", c=2 * CPT),
                in_=dstT[:, g * 2 * CPT:(g + 1) * 2 * CPT, None]
                    .to_broadcast([128, 2 * CPT, CHUNK]))
            M4 = pcv.tile([128, TE2], BF16, tag="M4")
            nc.vector.tensor_tensor(out=M4[:], in0=iotaF[:], in1=dstv[:],
                                    op=mybir.AluOpType.is_equal)

            ewb = pcv.tile([128, TE2], BF16, tag="ewb")
            for tl in range(2):
                ew4 = pmm.tile([128, TILE_E], F32, tag="ew4", space="PSUM")
                for c in range(CPT):
                    lsl = slice(tl * TILE_E + c * CHUNK,
                                tl * TILE_E + (c + 1) * CHUNK)
                    nc.tensor.matmul(out=ew4[:, c * CHUNK:(c + 1) * CHUNK],
                                     lhsT=B68[:, lsl], rhs=wtcomb[:],
                                     start=True, stop=True)
                if tl == 0:
                    nc.vector.tensor_copy(
                        out=ewb[:, tl * TILE_E:(tl + 1) * TILE_E], in_=ew4[:])
                else:
                    nc.scalar.activation(
                        out=ewb[:, tl * TILE_E:(tl + 1) * TILE_E], in_=ew4[:],
                        func=AF.Copy)
            hw4 = pcv.tile([128, TE2], BF16, tag="hw4")
            nc.vector.tensor_tensor(out=hw4[:], in0=hce[:, 0:TE2], in1=ewb[:],
                                    op=mybir.AluOpType.mult)

            # hs12 regions: [0:128]=t0_a [128:256]=t1_a
            #               [256:384]=t0_b [384:512]=t1_b
            hs12 = pacc.tile([128, 4 * 128], F32, tag="hs12", space="PSUM")
            cs = pacc.tile([128, 256], F32, tag="cs", space="PSUM")
            for tl in range(2):
                for lhs4, off, dst_ap in (
                        (hw4, 0, hs12[:, tl * 128:(tl + 1) * 128]),
                        (hce, 2 * TE2,
                         hs12[:, 256 + tl * 128:256 + (tl + 1) * 128]),
                        (hce, TE2, cs[:, tl * 128:(tl + 1) * 128])):
                    for c in range(CPT):
                        lo = tl * TILE_E + c * CHUNK
                        sl = slice(lo, lo + CHUNK)
                        nc.tensor.matmul(
                            out=dst_ap, lhsT=lhs4[:, off + lo:off + lo + CHUNK],
                            rhs=M4[:, sl], start=(c == 0),
                            stop=(c == CPT - 1))

            hsab = pnd.tile([128, 512], BF16, tag="hsab")
            nc.vector.tensor_copy(out=hsab[:], in_=hs12[:])
            css = pnd.tile([128, 256], F32, tag="css")
            nc.scalar.activation(out=css[:], in_=cs[:], func=AF.Copy)

            gate = {}
            for x, fn in (("f", "Sigmoid"), ("i", "Sigmoid"),
                          ("u", "Tanh"), ("o", "Sigmoid")):
                gp = pmm.tile([128, 256], F32, tag="mm", space="PSUM")
                nc.tensor.matmul(out=gp[:], lhsT=wg[x][0][:],
                                 rhs=hsab[:, 0:256], start=True, stop=False)
                nc.tensor.matmul(out=gp[:], lhsT=wg[x][1][:],
                                 rhs=hsab[:, 256:512], start=False, stop=True)
                gs = pnd.tile([128, 256], F32, tag="g_%s" % x)
                nc.scalar.activation(out=gs[:], in_=gp[:],
                                     func=getattr(AF, fn), bias=bias[x][:])
                gate[x] = gs

            # hc cols: [h(t0) h(t1) | c(t0) c(t1)] -- all 2D ops
            hc = pnd.tile([128, 512], BF16, tag="hc")
            ct = pnd.tile([128, 256], F32, tag="ct")
            nc.vector.tensor_tensor(out=ct[:], in0=gate["f"][:], in1=css[:],
                                    op=mybir.AluOpType.mult)
            iu = pnd.tile([128, 256], F32, tag="iu")
            nc.gpsimd.tensor_tensor(out=iu[:], in0=gate["i"][:],
                                    in1=gate["u"][:], op=mybir.AluOpType.mult)
            nc.vector.tensor_tensor(out=hc[:, 256:512], in0=iu[:], in1=ct[:],
                                    op=mybir.AluOpType.add)
            th = pnd.tile([128, 256], F32, tag="th")
            nc.scalar.activation(out=th[:], in_=hc[:, 256:512], func=AF.Tanh)
            nc.gpsimd.tensor_tensor(out=hc[:, 0:256], in0=gate["o"][:],
                                    in1=th[:], op=mybir.AluOpType.mult)
            nc.sync.dma_start(
                out=out_ext[:, g * 512:(g + 1) * 512], in_=hc[:])

        for p in (pmm, pacc, pnd, pcv, pin, cst):
            p.release()
    nc.finalize()
    return nc


def plan_tiles(dst_local, npc):
    """Greedy node tiling: <=128 nodes and <=TILE_E edges per tile.
    Returns list of (n0, n1, e0, e1) using sorted-edge offsets."""
    cnt = np.bincount(dst_local, minlength=npc)
    cum = np.concatenate([[0], np.cumsum(cnt)])
    tiles = []
    s = 0
    while s < npc:
        hi = min(s + 128, npc)
        m = int(np.searchsorted(cum, cum[s] + TILE_E, side="right")) - 1
        m = max(s + 1, min(hi, m))
        tiles.append((s, m, int(cum[s]), int(cum[m])))
        s = m
    return tiles


def prep_core(k, h_src, c_src, embed_dst, src_f, dst_f, etype, dst, T=None):
    """Build one core's padded, tiled input arrays."""
    lo = k * NPC
    sel = np.nonzero((dst >= lo) & (dst < lo + NPC))[0]
    dl = (dst[sel] - lo).astype(np.int64)
    order = np.argsort(dl, kind="stable")
    eidx = sel[order]
    dls = dl[order]
    tiles = plan_tiles(dls, NPC)
    Tk = len(tiles)
    if T is None:
        T = Tk
    assert Tk <= T
    ES = T * TILE_E
    src_slot = np.full(ES, -1, dtype=np.int64)
    dst_slot = np.full(ES, -1.0, dtype=np.float32)
    for t, (n0, n1, e0, e1) in enumerate(tiles):
        ne = e1 - e0
        assert ne <= TILE_E and n1 - n0 <= 128
        src_slot[t * TILE_E:t * TILE_E + ne] = eidx[e0:e1]
        dst_slot[t * TILE_E:t * TILE_E + ne] = (dls[e0:e1] - n0).astype(np.float32)
    val = src_slot >= 0
    gi = src_slot[val]

    def pad_rows(a, w):
        out = np.zeros((ES, w), dtype=np.float32)
        out[val] = a[gi]
        return out

    def chunk_layout(a):
        # [ES, H] -> [T, 128, TILE_E] with slot (c*128+p) at [t, p, c*128:...]
        return a.reshape(T, CPT, CHUNK, H).transpose(0, 2, 1, 3) \
                .reshape(T, 128, TILE_E)

    def pair(a):
        # [T,128,W] -> [T//2,128,2W] pairing consecutive tiles along cols
        Tn, P, W = a.shape
        return a.reshape(Tn // 2, 2, P, W).transpose(0, 2, 1, 3) \
                .reshape(Tn // 2, P, 2 * W)

    hp = pair(chunk_layout(pad_rows(h_src, H)))
    cp = pair(chunk_layout(pad_rows(c_src, H)))
    ep = pair(chunk_layout(pad_rows(embed_dst, H)))
    hce = np.ascontiguousarray(
        np.concatenate([hp, cp, ep], axis=2)).astype(bf16_np)
    sf = pair(pad_rows(src_f, G).reshape(T, TILE_E, G).transpose(0, 2, 1))
    df = pair(pad_rows(dst_f, G).reshape(T, TILE_E, G).transpose(0, 2, 1))
    sfdf = np.ascontiguousarray(
        np.concatenate([sf, df], axis=2)).astype(bf16_np)
    oh = np.zeros((ES, 4), dtype=np.float32)
    oh[val, etype[gi]] = 1.0
    oh[val, 3] = 1.0
    ohT = np.ascontiguousarray(oh.T).astype(bf16_np)
    dstT = np.ascontiguousarray(
        dst_slot.reshape(T * CPT, CHUNK).T).astype(bf16_np)
    return {"hce": hce, "sfdf": sfdf, "oh4": ohT, "dstT": dstT}, tiles, Tk


def _belB(b_el):
    out = np.zeros((4, 128), dtype=np.float32)
    out[3] = b_el
    return out


_graph_cache = {}


def kernel(**inputs):
    h_src = np.asarray(inputs["h_src"], dtype=np.float32)
    c_src = np.asarray(inputs["c_src"], dtype=np.float32)
    embed_dst = np.asarray(inputs["embed_dst"], dtype=np.float32)
    src_f = np.asarray(inputs["src_node_feat"], dtype=np.float32)
    dst_f = np.asarray(inputs["dst_node_feat"], dtype=np.float32)
    etype = np.asarray(inputs["edge_type_idx"]).astype(np.int64)
    dst = np.asarray(inputs["dst_idx"]).astype(np.int64)

    weights = {
        "welT": np.ascontiguousarray(np.asarray(inputs["W_el"], np.float32).T),
        "wa": np.ascontiguousarray(np.concatenate(
            [np.asarray(inputs["W_eoh"], np.float32),
             np.asarray(inputs["b_eoh"], np.float32)[:, None]], axis=1)),
        "belB": _belB(np.asarray(inputs["b_el"], np.float32)),
    }
    for x, wn, bwn, bn in (("f", "Wf", "bWf", "bf"), ("i", "Wi", "bWi", "bi"),
                           ("u", "Wu", "bWu", "bu"), ("o", "Wo", "bWo", "bo")):
        weights["w%sT" % x] = np.ascontiguousarray(
            np.asarray(inputs[wn], np.float32).T)
        weights["bW%s" % x] = np.asarray(inputs[bwn], np.float32)
        weights["b%s" % x] = np.asarray(inputs[bn], np.float32)

    planned = []
    for k in range(NCORES):
        lo = k * NPC
        sel = np.nonzero((dst >= lo) & (dst < lo + NPC))[0]
        dl = np.sort((dst[sel] - lo).astype(np.int64))
        planned.append(plan_tiles(dl, NPC))
    T = max(len(p) for p in planned)
    T += T % 2  # group-of-2 tiling needs even T

    in_maps = []
    tiles_all = []
    for k in range(NCORES):
        m, tiles, _ = prep_core(k, h_src, c_src, embed_dst, src_f, dst_f,
                                etype, dst, T=T)
        m.update(weights)
        in_maps.append(m)
        tiles_all.append(tiles)

    if T not in _graph_cache:
        _graph_cache[T] = build_graph(T)
    nc = _graph_cache[T]

    if TRACE:
        _install_axon_hook()
    res = run_bass_kernel_spmd(nc, in_maps, list(range(NCORES)), trace=TRACE)
    LAST["res"] = res

    out = np.empty((N, 2 * H), dtype=np.float32)
    for k in range(NCORES):
        outT = np.asarray(res.results[k]["outT"]).astype(np.float32)
        for t, (n0, n1, _, _) in enumerate(tiles_all[k]):
            nn = n1 - n0
            base = k * NPC
            gbase = (t // 2) * 512 + (t % 2) * 128
            out[base + n0:base + n1, 0:H] = outT[:, gbase:gbase + nn].T
            out[base + n0:base + n1, H:2 * H] = \
                outT[:, gbase + 256:gbase + 256 + nn].T
    return out


# revision 18
# speedup vs baseline: 1.4035x; 1.4035x over previous
"""ChildSumTreeLSTMCell on 8 Trainium2 NeuronCores.

Strategy: sort edges by destination node on the host (index preprocessing as
part of sharding), partition nodes contiguously across the 8 cores so every
core's segment sums are fully local -- zero collectives.  On each core, edges
are packed into node tiles (<=128 nodes, <=512 edges each, 4 chunks of 128
edge slots).  Segment sums become matmuls against a 0/1 membership matrix M
built on-device from the destination indices; the forget-gate gather
f[dst] * c_src factorizes to f * segment_sum(c_src), removing the second
scatter entirely.  All matmuls run in bf16 (inputs stay f32 in HBM).

v2: packed DMA streams (hce = h|c|e in one 768KB transfer, sfdf merged,
single merged [h|c] output), DMA issue spread across sync/gpsimd queues,
one-shot M build via free-dim broadcast, merged PSUM evacuation.
"""

import sys

for _p in ("/opt/trn_rl_repo", "/root/.axon_site/_ro/trn_rl_repo"):
    if _p not in sys.path:
        sys.path.append(_p)

import numpy as np
import ml_dtypes

import concourse.bacc as bacc
import concourse.mybir as mybir
import concourse.tile as tile
from concourse.bass_utils import run_bass_kernel_spmd

F32 = mybir.dt.float32
BF16 = mybir.dt.bfloat16

E = 500_000
N = 125_000
H = 128
G = 64
NCORES = 8
NPC = N // NCORES          # nodes per core
CHUNK = 128                # edges per chunk
CPT = 4                    # chunks per tile
TILE_E = CHUNK * CPT       # edge slots per tile
bf16_np = ml_dtypes.bfloat16

TRACE = False              # set by test.py to capture an NTFF profile
LAST = {}                  # last run's BassKernelResults


def _install_axon_hook():
    import types, contextlib, ctypes

    def _make_hook(so_path="/opt/axon/libaxon_pjrt.so"):
        lib = ctypes.CDLL(so_path)
        if not hasattr(lib, "axon_start_nrt_profile"):
            return None
        lib.axon_start_nrt_profile.argtypes = [
            ctypes.POINTER(ctypes.c_int64), ctypes.c_size_t]
        lib.axon_start_nrt_profile.restype = ctypes.c_int64
        lib.axon_stop_nrt_profile.argtypes = [ctypes.c_char_p]
        lib.axon_stop_nrt_profile.restype = ctypes.c_int64

        @contextlib.contextmanager
        def hook(output_dir, device_ids):
            import jax
            jax.devices()
            if device_ids:
                ids = (ctypes.c_int64 * len(device_ids))(*device_ids)
                rc = lib.axon_start_nrt_profile(ids, len(device_ids))
            else:
                rc = lib.axon_start_nrt_profile(None, 0)
            if rc != 0:
                raise RuntimeError("axon_start_nrt_profile rc=%d" % rc)
            try:
                yield
            finally:
                n = lib.axon_stop_nrt_profile(str(output_dir).encode())
                print("profile: %d file(s) written to %s" % (n, output_dir),
                      file=sys.stderr)

        return hook

    hook = _make_hook()
    mod = types.ModuleType("antenv.axon_hooks")
    mod.get_axon_ntff_profile_hook = lambda: hook
    mod.set_axon_ntff_profile_hook = lambda h: None
    sys.modules["antenv.axon_hooks"] = mod


def build_graph(T):
    """Build the per-core Bass graph for T node tiles."""
    nc = bacc.Bacc()
    dp = nc.declare_dram_parameter
    hce_ext = dp("hce", [T // 2, 128, 6 * TILE_E], BF16, isOutput=False)
    sfdf_ext = dp("sfdf", [T // 2, G, 4 * TILE_E], BF16, isOutput=False)
    oh4_ext = dp("oh4", [4, T * TILE_E], BF16, isOutput=False)
    dstT_ext = dp("dstT", [128, T * CPT], BF16, isOutput=False)
    welT_ext = dp("welT", [G, 128], F32, isOutput=False)
    wa_ext = dp("wa", [G, 4], F32, isOutput=False)
    bel_ext = dp("belB", [4, 128], F32, isOutput=False)
    wg_ext = {}
    for x in "fiuo":
        wg_ext[x] = dp("w%sT" % x, [2 * H, 128], F32, isOutput=False)
    bias_ext = {}
    for x in "fiuo":
        bias_ext[x] = (dp("bW%s" % x, [H], F32, isOutput=False),
                       dp("b%s" % x, [H], F32, isOutput=False))
    out_ext = dp("outT", [128, T * 2 * H], BF16, isOutput=True)

    with tile.TileContext(nc) as tc:
        cst = tc.alloc_tile_pool(name="cst", bufs=1)
        pin = tc.alloc_tile_pool(name="pin", bufs=8)
        pcv = tc.alloc_tile_pool(name="pcv", bufs=3)
        pnd = tc.alloc_tile_pool(name="pnd", bufs=3)
        pacc = tc.alloc_tile_pool(name="pacc", bufs=2, space="PSUM")
        pmm = tc.alloc_tile_pool(name="pmm", bufs=2, space="PSUM")

        # -- setup: constants -----------------------------------------------
        welT_sb = cst.tile([G, 128], F32)
        nc.sync.dma_start(out=welT_sb[:], in_=welT_ext[:])
        wa_sb = cst.tile([G, 4], F32)
        nc.sync.dma_start(out=wa_sb[:], in_=wa_ext[:])
        belr = cst.tile([4, 128], F32)
        nc.sync.dma_start(out=belr[:], in_=bel_ext[:])
        t2p = pmm.tile([4, 128], F32, tag="mm")
        nc.tensor.matmul(out=t2p[:], lhsT=wa_sb[:], rhs=welT_sb[:],
                         start=True, stop=True)
        wel_b16 = cst.tile([G, 128], BF16)
        nc.vector.tensor_copy(out=wel_b16[:], in_=welT_sb[:])
        t4b = cst.tile([4, 128], BF16)
        nc.vector.tensor_tensor(out=t4b[:], in0=t2p[:],
                                in1=belr[:], op=mybir.AluOpType.add)
        wtcomb = cst.tile([G + 4, 128], BF16)
        nc.sync.dma_start(out=wtcomb[0:G, :], in_=wel_b16[:])
        nc.sync.dma_start(out=wtcomb[G:G + 4, :], in_=t4b[:])

        wg = {}
        for x in "fiuo":
            stg = cst.tile([128, 128], F32, tag="wstg_%s" % x)
            nc.sync.dma_start(out=stg[:], in_=wg_ext[x][0:128, :])
            wa_t = cst.tile([128, 128], BF16, tag="wg_%s_a" % x)
            nc.vector.tensor_copy(out=wa_t[:], in_=stg[:])
            stg2 = cst.tile([128, 128], F32, tag="wstg2_%s" % x)
            nc.sync.dma_start(out=stg2[:], in_=wg_ext[x][128:256, :])
            wb_t = cst.tile([128, 128], BF16, tag="wg_%s_b" % x)
            nc.vector.tensor_copy(out=wb_t[:], in_=stg2[:])
            wg[x] = (wa_t, wb_t)

        bias = {}
        for x in "fiuo":
            b1 = cst.tile([128, 1], F32, tag="b1_%s" % x)
            nc.sync.dma_start(out=b1[:], in_=bias_ext[x][0][:, None])
            b2 = cst.tile([128, 1], F32, tag="b2_%s" % x)
            nc.sync.dma_start(out=b2[:], in_=bias_ext[x][1][:, None])
            bs = cst.tile([128, 1], F32, tag="bs_%s" % x)
            nc.vector.tensor_tensor(out=bs[:], in0=b1[:], in1=b2[:],
                                    op=mybir.AluOpType.add)
            bias[x] = bs

        it32 = cst.tile([128, 2 * TILE_E], mybir.dt.int32)
        nc.gpsimd.iota(out=it32[:], pattern=[[0, 2 * CPT], [1, CHUNK]], base=0,
                       channel_multiplier=0)
        iotaF = cst.tile([128, 2 * TILE_E], BF16)
        nc.vector.tensor_copy(out=iotaF[:], in_=it32[:])

        dstT = cst.tile([128, T * CPT], BF16)
        nc.sync.dma_start(out=dstT[:], in_=dstT_ext[:])

        # -- main loop: groups of 2 node tiles ------------------------------
        # hce cols (bf16): [h(t0) h(t1) | c(t0) c(t1) | e(t0) e(t1)]
        # sfdf cols (bf16): [sf(t0) sf(t1) | df(t0) df(t1)]
        # hc cols (f32):  [h(t0) h(t1) | c(t0) c(t1)] (host unmaps)
        assert T % 2 == 0
        AF = mybir.ActivationFunctionType
        TE2 = 2 * TILE_E
        for g in range(T // 2):
            hce = pin.tile([128, 3 * TE2], BF16, tag="hce")
            nc.sync.dma_start(out=hce[:], in_=hce_ext[g])
            sfdf = pin.tile([G, 2 * TE2], BF16, tag="sfdf")
            nc.scalar.dma_start(out=sfdf[:], in_=sfdf_ext[g])
            B68 = pcv.tile([G + 4, TE2], BF16, tag="B68")
            nc.sync.dma_start(
                out=B68[G:G + 4, :],
                in_=oh4_ext[:, g * TE2:(g + 1) * TE2])
            nc.vector.tensor_tensor(
                out=B68[0:G, :], in0=sfdf[:, 0:TE2], in1=sfdf[:, TE2:2 * TE2],
                op=mybir.AluOpType.mult)
            M4 = pcv.tile([128, TE2], BF16, tag="M4")
            nc.vector.tensor_tensor(
                out=M4[:].rearrange("p (c j) -> p c j", c=2 * CPT),
                in0=iotaF[:].rearrange("p (c j) -> p c j", c=2 * CPT),
                in1=dstT[:, g * 2 * CPT:(g + 1) * 2 * CPT, None]
                    .to_broadcast([128, 2 * CPT, CHUNK]),
                op=mybir.AluOpType.is_equal)

            ewb = pcv.tile([128, TE2], BF16, tag="ewb")
            for tl in range(2):
                ew4 = pmm.tile([128, TILE_E], F32, tag="ew4", space="PSUM")
                for c in range(CPT):
                    lsl = slice(tl * TILE_E + c * CHUNK,
                                tl * TILE_E + (c + 1) * CHUNK)
                    nc.tensor.matmul(out=ew4[:, c * CHUNK:(c + 1) * CHUNK],
                                     lhsT=B68[:, lsl], rhs=wtcomb[:],
                                     start=True, stop=True)
                if tl == 0:
                    nc.vector.tensor_copy(
                        out=ewb[:, tl * TILE_E:(tl + 1) * TILE_E], in_=ew4[:])
                else:
                    nc.scalar.activation(
                        out=ewb[:, tl * TILE_E:(tl + 1) * TILE_E], in_=ew4[:],
                        func=AF.Copy)
            hw4 = pcv.tile([128, TE2], BF16, tag="hw4")
            nc.vector.tensor_tensor(out=hw4[:], in0=hce[:, 0:TE2], in1=ewb[:],
                                    op=mybir.AluOpType.mult)

            # hs12 regions: [0:128]=t0_a [128:256]=t1_a
            #               [256:384]=t0_b [384:512]=t1_b
            hs12 = pacc.tile([128, 4 * 128], F32, tag="hs12", space="PSUM")
            cs = pacc.tile([128, 256], F32, tag="cs", space="PSUM")
            for tl in range(2):
                for lhs4, off, dst_ap in (
                        (hw4, 0, hs12[:, tl * 128:(tl + 1) * 128]),
                        (hce, 2 * TE2,
                         hs12[:, 256 + tl * 128:256 + (tl + 1) * 128]),
                        (hce, TE2, cs[:, tl * 128:(tl + 1) * 128])):
                    for c in range(CPT):
                        lo = tl * TILE_E + c * CHUNK
                        sl = slice(lo, lo + CHUNK)
                        nc.tensor.matmul(
                            out=dst_ap, lhsT=lhs4[:, off + lo:off + lo + CHUNK],
                            rhs=M4[:, sl], start=(c == 0),
                            stop=(c == CPT - 1))

            hsab = pnd.tile([128, 512], BF16, tag="hsab")
            nc.vector.tensor_copy(out=hsab[:], in_=hs12[:])
            css = pnd.tile([128, 256], F32, tag="css")
            nc.scalar.activation(out=css[:], in_=cs[:], func=AF.Copy)

            gate = {}
            for x, fn in (("f", "Sigmoid"), ("i", "Sigmoid"),
                          ("u", "Tanh"), ("o", "Sigmoid")):
                gp = pmm.tile([128, 256], F32, tag="mm", space="PSUM")
                nc.tensor.matmul(out=gp[:], lhsT=wg[x][0][:],
                                 rhs=hsab[:, 0:256], start=True, stop=False)
                nc.tensor.matmul(out=gp[:], lhsT=wg[x][1][:],
                                 rhs=hsab[:, 256:512], start=False, stop=True)
                gs = pnd.tile([128, 256], F32, tag="g_%s" % x)
                nc.scalar.activation(out=gs[:], in_=gp[:],
                                     func=getattr(AF, fn), bias=bias[x][:])
                gate[x] = gs

            # hc cols: [h(t0) h(t1) | c(t0) c(t1)] -- all 2D ops
            hc = pnd.tile([128, 512], BF16, tag="hc")
            ct = pnd.tile([128, 256], F32, tag="ct")
            nc.vector.tensor_tensor(out=ct[:], in0=gate["f"][:], in1=css[:],
                                    op=mybir.AluOpType.mult)
            iu = pnd.tile([128, 256], F32, tag="iu")
            nc.gpsimd.tensor_tensor(out=iu[:], in0=gate["i"][:],
                                    in1=gate["u"][:], op=mybir.AluOpType.mult)
            nc.vector.tensor_tensor(out=hc[:, 256:512], in0=iu[:], in1=ct[:],
                                    op=mybir.AluOpType.add)
            th = pnd.tile([128, 256], F32, tag="th")
            nc.scalar.activation(out=th[:], in_=hc[:, 256:512], func=AF.Tanh)
            nc.gpsimd.tensor_tensor(out=hc[:, 0:256], in0=gate["o"][:],
                                    in1=th[:], op=mybir.AluOpType.mult)
            nc.sync.dma_start(
                out=out_ext[:, g * 512:(g + 1) * 512], in_=hc[:])

        for p in (pmm, pacc, pnd, pcv, pin, cst):
            p.release()
    nc.finalize()
    return nc


def plan_tiles(dst_local, npc):
    """Greedy node tiling: <=128 nodes and <=TILE_E edges per tile.
    Returns list of (n0, n1, e0, e1) using sorted-edge offsets."""
    cnt = np.bincount(dst_local, minlength=npc)
    cum = np.concatenate([[0], np.cumsum(cnt)])
    tiles = []
    s = 0
    while s < npc:
        hi = min(s + 128, npc)
        m = int(np.searchsorted(cum, cum[s] + TILE_E, side="right")) - 1
        m = max(s + 1, min(hi, m))
        tiles.append((s, m, int(cum[s]), int(cum[m])))
        s = m
    return tiles


def prep_core(k, h_src, c_src, embed_dst, src_f, dst_f, etype, dst, T=None):
    """Build one core's padded, tiled input arrays."""
    lo = k * NPC
    sel = np.nonzero((dst >= lo) & (dst < lo + NPC))[0]
    dl = (dst[sel] - lo).astype(np.int64)
    order = np.argsort(dl, kind="stable")
    eidx = sel[order]
    dls = dl[order]
    tiles = plan_tiles(dls, NPC)
    Tk = len(tiles)
    if T is None:
        T = Tk
    assert Tk <= T
    ES = T * TILE_E
    src_slot = np.full(ES, -1, dtype=np.int64)
    dst_slot = np.full(ES, -1.0, dtype=np.float32)
    for t, (n0, n1, e0, e1) in enumerate(tiles):
        ne = e1 - e0
        assert ne <= TILE_E and n1 - n0 <= 128
        src_slot[t * TILE_E:t * TILE_E + ne] = eidx[e0:e1]
        dst_slot[t * TILE_E:t * TILE_E + ne] = (dls[e0:e1] - n0).astype(np.float32)
    val = src_slot >= 0
    gi = src_slot[val]

    def pad_rows(a, w):
        out = np.zeros((ES, w), dtype=np.float32)
        out[val] = a[gi]
        return out

    def chunk_layout(a):
        # [ES, H] -> [T, 128, TILE_E] with slot (c*128+p) at [t, p, c*128:...]
        return a.reshape(T, CPT, CHUNK, H).transpose(0, 2, 1, 3) \
                .reshape(T, 128, TILE_E)

    def pair(a):
        # [T,128,W] -> [T//2,128,2W] pairing consecutive tiles along cols
        Tn, P, W = a.shape
        return a.reshape(Tn // 2, 2, P, W).transpose(0, 2, 1, 3) \
                .reshape(Tn // 2, P, 2 * W)

    hp = pair(chunk_layout(pad_rows(h_src, H)))
    cp = pair(chunk_layout(pad_rows(c_src, H)))
    ep = pair(chunk_layout(pad_rows(embed_dst, H)))
    hce = np.ascontiguousarray(
        np.concatenate([hp, cp, ep], axis=2)).astype(bf16_np)
    sf = pair(pad_rows(src_f, G).reshape(T, TILE_E, G).transpose(0, 2, 1))
    df = pair(pad_rows(dst_f, G).reshape(T, TILE_E, G).transpose(0, 2, 1))
    sfdf = np.ascontiguousarray(
        np.concatenate([sf, df], axis=2)).astype(bf16_np)
    oh = np.zeros((ES, 4), dtype=np.float32)
    oh[val, etype[gi]] = 1.0
    oh[val, 3] = 1.0
    ohT = np.ascontiguousarray(oh.T).astype(bf16_np)
    dstT = np.ascontiguousarray(
        dst_slot.reshape(T * CPT, CHUNK).T).astype(bf16_np)
    return {"hce": hce, "sfdf": sfdf, "oh4": ohT, "dstT": dstT}, tiles, Tk


def _belB(b_el):
    out = np.zeros((4, 128), dtype=np.float32)
    out[3] = b_el
    return out


_graph_cache = {}


def kernel(**inputs):
    h_src = np.asarray(inputs["h_src"], dtype=np.float32)
    c_src = np.asarray(inputs["c_src"], dtype=np.float32)
    embed_dst = np.asarray(inputs["embed_dst"], dtype=np.float32)
    src_f = np.asarray(inputs["src_node_feat"], dtype=np.float32)
    dst_f = np.asarray(inputs["dst_node_feat"], dtype=np.float32)
    etype = np.asarray(inputs["edge_type_idx"]).astype(np.int64)
    dst = np.asarray(inputs["dst_idx"]).astype(np.int64)

    weights = {
        "welT": np.ascontiguousarray(np.asarray(inputs["W_el"], np.float32).T),
        "wa": np.ascontiguousarray(np.concatenate(
            [np.asarray(inputs["W_eoh"], np.float32),
             np.asarray(inputs["b_eoh"], np.float32)[:, None]], axis=1)),
        "belB": _belB(np.asarray(inputs["b_el"], np.float32)),
    }
    for x, wn, bwn, bn in (("f", "Wf", "bWf", "bf"), ("i", "Wi", "bWi", "bi"),
                           ("u", "Wu", "bWu", "bu"), ("o", "Wo", "bWo", "bo")):
        weights["w%sT" % x] = np.ascontiguousarray(
            np.asarray(inputs[wn], np.float32).T)
        weights["bW%s" % x] = np.asarray(inputs[bwn], np.float32)
        weights["b%s" % x] = np.asarray(inputs[bn], np.float32)

    planned = []
    for k in range(NCORES):
        lo = k * NPC
        sel = np.nonzero((dst >= lo) & (dst < lo + NPC))[0]
        dl = np.sort((dst[sel] - lo).astype(np.int64))
        planned.append(plan_tiles(dl, NPC))
    T = max(len(p) for p in planned)
    T += T % 2  # group-of-2 tiling needs even T

    in_maps = []
    tiles_all = []
    for k in range(NCORES):
        m, tiles, _ = prep_core(k, h_src, c_src, embed_dst, src_f, dst_f,
                                etype, dst, T=T)
        m.update(weights)
        in_maps.append(m)
        tiles_all.append(tiles)

    if T not in _graph_cache:
        _graph_cache[T] = build_graph(T)
    nc = _graph_cache[T]

    if TRACE:
        _install_axon_hook()
    res = run_bass_kernel_spmd(nc, in_maps, list(range(NCORES)), trace=TRACE)
    LAST["res"] = res

    out = np.empty((N, 2 * H), dtype=np.float32)
    for k in range(NCORES):
        outT = np.asarray(res.results[k]["outT"]).astype(np.float32)
        for t, (n0, n1, _, _) in enumerate(tiles_all[k]):
            nn = n1 - n0
            base = k * NPC
            gbase = (t // 2) * 512 + (t % 2) * 128
            out[base + n0:base + n1, 0:H] = outT[:, gbase:gbase + nn].T
            out[base + n0:base + n1, H:2 * H] = \
                outT[:, gbase + 256:gbase + 256 + nn].T
    return out


# revision 19
# speedup vs baseline: 1.5099x; 1.0758x over previous
"""ChildSumTreeLSTMCell on 8 Trainium2 NeuronCores.

Strategy: sort edges by destination node on the host (index preprocessing as
part of sharding), partition nodes contiguously across the 8 cores so every
core's segment sums are fully local -- zero collectives.  On each core, edges
are packed into node tiles (<=128 nodes, <=512 edges each, 4 chunks of 128
edge slots).  Segment sums become matmuls against a 0/1 membership matrix M
built on-device from the destination indices; the forget-gate gather
f[dst] * c_src factorizes to f * segment_sum(c_src), removing the second
scatter entirely.  All matmuls run in bf16 (inputs stay f32 in HBM).

v2: packed DMA streams (hce = h|c|e in one 768KB transfer, sfdf merged,
single merged [h|c] output), DMA issue spread across sync/gpsimd queues,
one-shot M build via free-dim broadcast, merged PSUM evacuation.
"""

import sys

for _p in ("/opt/trn_rl_repo", "/root/.axon_site/_ro/trn_rl_repo"):
    if _p not in sys.path:
        sys.path.append(_p)

import numpy as np
import ml_dtypes

import concourse.bacc as bacc
import concourse.mybir as mybir
import concourse.tile as tile
from concourse.bass_utils import run_bass_kernel_spmd

F32 = mybir.dt.float32
BF16 = mybir.dt.bfloat16

E = 500_000
N = 125_000
H = 128
G = 64
NCORES = 8
NPC = N // NCORES          # nodes per core
CHUNK = 128                # edges per chunk
CPT = 4                    # chunks per tile
TILE_E = CHUNK * CPT       # edge slots per tile
GRP = 4                    # node tiles per device loop group
bf16_np = ml_dtypes.bfloat16

TRACE = False              # set by test.py to capture an NTFF profile
LAST = {}                  # last run's BassKernelResults


def _install_axon_hook():
    import types, contextlib, ctypes

    def _make_hook(so_path="/opt/axon/libaxon_pjrt.so"):
        lib = ctypes.CDLL(so_path)
        if not hasattr(lib, "axon_start_nrt_profile"):
            return None
        lib.axon_start_nrt_profile.argtypes = [
            ctypes.POINTER(ctypes.c_int64), ctypes.c_size_t]
        lib.axon_start_nrt_profile.restype = ctypes.c_int64
        lib.axon_stop_nrt_profile.argtypes = [ctypes.c_char_p]
        lib.axon_stop_nrt_profile.restype = ctypes.c_int64

        @contextlib.contextmanager
        def hook(output_dir, device_ids):
            import jax
            jax.devices()
            if device_ids:
                ids = (ctypes.c_int64 * len(device_ids))(*device_ids)
                rc = lib.axon_start_nrt_profile(ids, len(device_ids))
            else:
                rc = lib.axon_start_nrt_profile(None, 0)
            if rc != 0:
                raise RuntimeError("axon_start_nrt_profile rc=%d" % rc)
            try:
                yield
            finally:
                n = lib.axon_stop_nrt_profile(str(output_dir).encode())
                print("profile: %d file(s) written to %s" % (n, output_dir),
                      file=sys.stderr)

        return hook

    hook = _make_hook()
    mod = types.ModuleType("antenv.axon_hooks")
    mod.get_axon_ntff_profile_hook = lambda: hook
    mod.set_axon_ntff_profile_hook = lambda h: None
    sys.modules["antenv.axon_hooks"] = mod


def build_graph(T):
    """Build the per-core Bass graph for T node tiles."""
    nc = bacc.Bacc()
    dp = nc.declare_dram_parameter
    hce_ext = dp("hce", [T // GRP, 128, 3 * GRP * TILE_E], BF16, isOutput=False)
    sfdf_ext = dp("sfdf", [T // GRP, G, 2 * GRP * TILE_E], BF16, isOutput=False)
    oh4_ext = dp("oh4", [4, T * TILE_E], BF16, isOutput=False)
    dstT_ext = dp("dstT", [128, T * CPT], BF16, isOutput=False)
    welT_ext = dp("welT", [G, 128], F32, isOutput=False)
    wa_ext = dp("wa", [G, 4], F32, isOutput=False)
    bel_ext = dp("belB", [4, 128], F32, isOutput=False)
    wg_ext = {}
    for x in "fiuo":
        wg_ext[x] = dp("w%sT" % x, [2 * H, 128], F32, isOutput=False)
    bias_ext = {}
    for x in "fiuo":
        bias_ext[x] = (dp("bW%s" % x, [H], F32, isOutput=False),
                       dp("b%s" % x, [H], F32, isOutput=False))
    out_ext = dp("outT", [128, T * 2 * H], BF16, isOutput=True)

    with tile.TileContext(nc) as tc:
        cst = tc.alloc_tile_pool(name="cst", bufs=1)
        pin = tc.alloc_tile_pool(name="pin", bufs=4)
        pcv = tc.alloc_tile_pool(name="pcv", bufs=2)
        pnd = tc.alloc_tile_pool(name="pnd", bufs=2)
        pacc = tc.alloc_tile_pool(name="pacc", bufs=1, space="PSUM")
        pmm = tc.alloc_tile_pool(name="pmm", bufs=2, space="PSUM")

        # -- setup: constants -----------------------------------------------
        welT_sb = cst.tile([G, 128], F32)
        nc.sync.dma_start(out=welT_sb[:], in_=welT_ext[:])
        wa_sb = cst.tile([G, 4], F32)
        nc.sync.dma_start(out=wa_sb[:], in_=wa_ext[:])
        belr = cst.tile([4, 128], F32)
        nc.sync.dma_start(out=belr[:], in_=bel_ext[:])
        t2p = pmm.tile([4, 128], F32, tag="mm")
        nc.tensor.matmul(out=t2p[:], lhsT=wa_sb[:], rhs=welT_sb[:],
                         start=True, stop=True)
        wel_b16 = cst.tile([G, 128], BF16)
        nc.vector.tensor_copy(out=wel_b16[:], in_=welT_sb[:])
        t4b = cst.tile([4, 128], BF16)
        nc.vector.tensor_tensor(out=t4b[:], in0=t2p[:],
                                in1=belr[:], op=mybir.AluOpType.add)
        wtcomb = cst.tile([G + 4, 128], BF16)
        nc.sync.dma_start(out=wtcomb[0:G, :], in_=wel_b16[:])
        nc.sync.dma_start(out=wtcomb[G:G + 4, :], in_=t4b[:])

        wg = {}
        for x in "fiuo":
            stg = cst.tile([128, 128], F32, tag="wstg_%s" % x)
            nc.sync.dma_start(out=stg[:], in_=wg_ext[x][0:128, :])
            wa_t = cst.tile([128, 128], BF16, tag="wg_%s_a" % x)
            nc.vector.tensor_copy(out=wa_t[:], in_=stg[:])
            stg2 = cst.tile([128, 128], F32, tag="wstg2_%s" % x)
            nc.sync.dma_start(out=stg2[:], in_=wg_ext[x][128:256, :])
            wb_t = cst.tile([128, 128], BF16, tag="wg_%s_b" % x)
            nc.vector.tensor_copy(out=wb_t[:], in_=stg2[:])
            wg[x] = (wa_t, wb_t)

        bias = {}
        for x in "fiuo":
            b1 = cst.tile([128, 1], F32, tag="b1_%s" % x)
            nc.sync.dma_start(out=b1[:], in_=bias_ext[x][0][:, None])
            b2 = cst.tile([128, 1], F32, tag="b2_%s" % x)
            nc.sync.dma_start(out=b2[:], in_=bias_ext[x][1][:, None])
            bs = cst.tile([128, 1], F32, tag="bs_%s" % x)
            nc.vector.tensor_tensor(out=bs[:], in0=b1[:], in1=b2[:],
                                    op=mybir.AluOpType.add)
            bias[x] = bs

        it32 = cst.tile([128, GRP * TILE_E], mybir.dt.int32)
        nc.gpsimd.iota(out=it32[:], pattern=[[0, GRP * CPT], [1, CHUNK]],
                       base=0, channel_multiplier=0)
        iotaF = cst.tile([128, GRP * TILE_E], BF16)
        nc.vector.tensor_copy(out=iotaF[:], in_=it32[:])

        dstT = cst.tile([128, T * CPT], BF16)
        nc.sync.dma_start(out=dstT[:], in_=dstT_ext[:])

        # -- main loop: groups of GRP node tiles ----------------------------
        # hce cols (bf16): [h x GRP tiles | c x GRP | e x GRP]
        # sfdf cols (bf16): [sf x GRP | df x GRP]
        # hc cols (bf16):  [h x GRP | c x GRP] (host unmaps)
        assert T % GRP == 0
        AF = mybir.ActivationFunctionType
        TEG = GRP * TILE_E
        NG = GRP * 128            # node slots per group
        for g in range(T // GRP):
            hce = pin.tile([128, 3 * TEG], BF16, tag="hce")
            nc.sync.dma_start(out=hce[:], in_=hce_ext[g])
            sfdf = pin.tile([G, 2 * TEG], BF16, tag="sfdf")
            nc.scalar.dma_start(out=sfdf[:], in_=sfdf_ext[g])
            B68 = pcv.tile([G + 4, TEG], BF16, tag="B68")
            nc.sync.dma_start(
                out=B68[G:G + 4, :],
                in_=oh4_ext[:, g * TEG:(g + 1) * TEG])
            nc.vector.tensor_tensor(
                out=B68[0:G, :], in0=sfdf[:, 0:TEG], in1=sfdf[:, TEG:2 * TEG],
                op=mybir.AluOpType.mult)
            M4 = pcv.tile([128, TEG], BF16, tag="M4")
            nc.vector.tensor_tensor(
                out=M4[:].rearrange("p (c j) -> p c j", c=GRP * CPT),
                in0=iotaF[:].rearrange("p (c j) -> p c j", c=GRP * CPT),
                in1=dstT[:, g * GRP * CPT:(g + 1) * GRP * CPT, None]
                    .to_broadcast([128, GRP * CPT, CHUNK]),
                op=mybir.AluOpType.is_equal)

            ewb = pcv.tile([128, TEG], BF16, tag="ewb")
            for tl in range(GRP):
                ew4 = pmm.tile([128, TILE_E], F32, tag="ew4", space="PSUM")
                for c in range(CPT):
                    lsl = slice(tl * TILE_E + c * CHUNK,
                                tl * TILE_E + (c + 1) * CHUNK)
                    nc.tensor.matmul(out=ew4[:, c * CHUNK:(c + 1) * CHUNK],
                                     lhsT=B68[:, lsl], rhs=wtcomb[:],
                                     start=True, stop=True)
                if tl % 2 == 0:
                    nc.vector.tensor_copy(
                        out=ewb[:, tl * TILE_E:(tl + 1) * TILE_E], in_=ew4[:])
                else:
                    nc.scalar.activation(
                        out=ewb[:, tl * TILE_E:(tl + 1) * TILE_E], in_=ew4[:],
                        func=AF.Copy)
            hw4 = pcv.tile([128, TEG], BF16, tag="hw4")
            nc.vector.tensor_tensor(out=hw4[:], in0=hce[:, 0:TEG], in1=ewb[:],
                                    op=mybir.AluOpType.mult)

            # hs12 regions: a-part cols [tl*128], b-part cols [NG + tl*128]
            hs12 = pacc.tile([128, 2 * NG], F32, tag="hs12", space="PSUM")
            cs = pacc.tile([128, NG], F32, tag="cs", space="PSUM")
            for tl in range(GRP):
                for lhs4, off, dst_ap in (
                        (hw4, 0, hs12[:, tl * 128:(tl + 1) * 128]),
                        (hce, 2 * TEG,
                         hs12[:, NG + tl * 128:NG + (tl + 1) * 128]),
                        (hce, TEG, cs[:, tl * 128:(tl + 1) * 128])):
                    for c in range(CPT):
                        lo = tl * TILE_E + c * CHUNK
                        nc.tensor.matmul(
                            out=dst_ap, lhsT=lhs4[:, off + lo:off + lo + CHUNK],
                            rhs=M4[:, lo:lo + CHUNK], start=(c == 0),
                            stop=(c == CPT - 1))

            hsab = pnd.tile([128, 2 * NG], BF16, tag="hsab")
            nc.vector.tensor_copy(out=hsab[:], in_=hs12[:])
            css = pnd.tile([128, NG], F32, tag="css")
            nc.scalar.activation(out=css[:], in_=cs[:], func=AF.Copy)

            gate = {}
            for x, fn in (("f", "Sigmoid"), ("i", "Sigmoid"),
                          ("u", "Tanh"), ("o", "Sigmoid")):
                gp = pmm.tile([128, NG], F32, tag="mm", space="PSUM")
                nc.tensor.matmul(out=gp[:], lhsT=wg[x][0][:],
                                 rhs=hsab[:, 0:NG], start=True, stop=False)
                nc.tensor.matmul(out=gp[:], lhsT=wg[x][1][:],
                                 rhs=hsab[:, NG:2 * NG], start=False, stop=True)
                gs = pnd.tile([128, NG], F32, tag="g_%s" % x)
                nc.scalar.activation(out=gs[:], in_=gp[:],
                                     func=getattr(AF, fn), bias=bias[x][:])
                gate[x] = gs

            # hc cols: [h x GRP | c x GRP] -- all 2D ops
            hc = pnd.tile([128, 2 * NG], BF16, tag="hc")
            ct = pnd.tile([128, NG], F32, tag="ct")
            nc.vector.tensor_tensor(out=ct[:], in0=gate["f"][:], in1=css[:],
                                    op=mybir.AluOpType.mult)
            iu = pnd.tile([128, NG], F32, tag="iu")
            nc.gpsimd.tensor_tensor(out=iu[:], in0=gate["i"][:],
                                    in1=gate["u"][:], op=mybir.AluOpType.mult)
            nc.vector.tensor_tensor(out=hc[:, NG:2 * NG], in0=iu[:], in1=ct[:],
                                    op=mybir.AluOpType.add)
            th = pnd.tile([128, NG], F32, tag="th")
            nc.scalar.activation(out=th[:], in_=hc[:, NG:2 * NG], func=AF.Tanh)
            nc.gpsimd.tensor_tensor(out=hc[:, 0:NG], in0=gate["o"][:],
                                    in1=th[:], op=mybir.AluOpType.mult)
            nc.sync.dma_start(
                out=out_ext[:, g * 2 * NG:(g + 1) * 2 * NG], in_=hc[:])

        for p in (pmm, pacc, pnd, pcv, pin, cst):
            p.release()
    nc.finalize()
    return nc


def plan_tiles(dst_local, npc):
    """Greedy node tiling: <=128 nodes and <=TILE_E edges per tile.
    Returns list of (n0, n1, e0, e1) using sorted-edge offsets."""
    cnt = np.bincount(dst_local, minlength=npc)
    cum = np.concatenate([[0], np.cumsum(cnt)])
    tiles = []
    s = 0
    while s < npc:
        hi = min(s + 128, npc)
        m = int(np.searchsorted(cum, cum[s] + TILE_E, side="right")) - 1
        m = max(s + 1, min(hi, m))
        tiles.append((s, m, int(cum[s]), int(cum[m])))
        s = m
    return tiles


def prep_core(k, h_src, c_src, embed_dst, src_f, dst_f, etype, dst, T=None):
    """Build one core's padded, tiled input arrays."""
    lo = k * NPC
    sel = np.nonzero((dst >= lo) & (dst < lo + NPC))[0]
    dl = (dst[sel] - lo).astype(np.int64)
    order = np.argsort(dl, kind="stable")
    eidx = sel[order]
    dls = dl[order]
    tiles = plan_tiles(dls, NPC)
    Tk = len(tiles)
    if T is None:
        T = Tk
    assert Tk <= T
    ES = T * TILE_E
    src_slot = np.full(ES, -1, dtype=np.int64)
    dst_slot = np.full(ES, -1.0, dtype=np.float32)
    for t, (n0, n1, e0, e1) in enumerate(tiles):
        ne = e1 - e0
        assert ne <= TILE_E and n1 - n0 <= 128
        src_slot[t * TILE_E:t * TILE_E + ne] = eidx[e0:e1]
        dst_slot[t * TILE_E:t * TILE_E + ne] = (dls[e0:e1] - n0).astype(np.float32)
    val = src_slot >= 0
    gi = src_slot[val]

    def pad_rows(a, w):
        out = np.zeros((ES, w), dtype=np.float32)
        out[val] = a[gi]
        return out

    def chunk_layout(a):
        # [ES, H] -> [T, 128, TILE_E] with slot (c*128+p) at [t, p, c*128:...]
        return a.reshape(T, CPT, CHUNK, H).transpose(0, 2, 1, 3) \
                .reshape(T, 128, TILE_E)

    def pair(a):
        # [T,128,W] -> [T//GRP,128,GRP*W] grouping consecutive tiles
        Tn, P, W = a.shape
        return a.reshape(Tn // GRP, GRP, P, W).transpose(0, 2, 1, 3) \
                .reshape(Tn // GRP, P, GRP * W)

    hp = pair(chunk_layout(pad_rows(h_src, H)))
    cp = pair(chunk_layout(pad_rows(c_src, H)))
    ep = pair(chunk_layout(pad_rows(embed_dst, H)))
    hce = np.ascontiguousarray(
        np.concatenate([hp, cp, ep], axis=2)).astype(bf16_np)
    sf = pair(pad_rows(src_f, G).reshape(T, TILE_E, G).transpose(0, 2, 1))
    df = pair(pad_rows(dst_f, G).reshape(T, TILE_E, G).transpose(0, 2, 1))
    sfdf = np.ascontiguousarray(
        np.concatenate([sf, df], axis=2)).astype(bf16_np)
    oh = np.zeros((ES, 4), dtype=np.float32)
    oh[val, etype[gi]] = 1.0
    oh[val, 3] = 1.0
    ohT = np.ascontiguousarray(oh.T).astype(bf16_np)
    dstT = np.ascontiguousarray(
        dst_slot.reshape(T * CPT, CHUNK).T).astype(bf16_np)
    return {"hce": hce, "sfdf": sfdf, "oh4": ohT, "dstT": dstT}, tiles, Tk


def _belB(b_el):
    out = np.zeros((4, 128), dtype=np.float32)
    out[3] = b_el
    return out


_graph_cache = {}


def kernel(**inputs):
    h_src = np.asarray(inputs["h_src"], dtype=np.float32)
    c_src = np.asarray(inputs["c_src"], dtype=np.float32)
    embed_dst = np.asarray(inputs["embed_dst"], dtype=np.float32)
    src_f = np.asarray(inputs["src_node_feat"], dtype=np.float32)
    dst_f = np.asarray(inputs["dst_node_feat"], dtype=np.float32)
    etype = np.asarray(inputs["edge_type_idx"]).astype(np.int64)
    dst = np.asarray(inputs["dst_idx"]).astype(np.int64)

    weights = {
        "welT": np.ascontiguousarray(np.asarray(inputs["W_el"], np.float32).T),
        "wa": np.ascontiguousarray(np.concatenate(
            [np.asarray(inputs["W_eoh"], np.float32),
             np.asarray(inputs["b_eoh"], np.float32)[:, None]], axis=1)),
        "belB": _belB(np.asarray(inputs["b_el"], np.float32)),
    }
    for x, wn, bwn, bn in (("f", "Wf", "bWf", "bf"), ("i", "Wi", "bWi", "bi"),
                           ("u", "Wu", "bWu", "bu"), ("o", "Wo", "bWo", "bo")):
        weights["w%sT" % x] = np.ascontiguousarray(
            np.asarray(inputs[wn], np.float32).T)
        weights["bW%s" % x] = np.asarray(inputs[bwn], np.float32)
        weights["b%s" % x] = np.asarray(inputs[bn], np.float32)

    planned = []
    for k in range(NCORES):
        lo = k * NPC
        sel = np.nonzero((dst >= lo) & (dst < lo + NPC))[0]
        dl = np.sort((dst[sel] - lo).astype(np.int64))
        planned.append(plan_tiles(dl, NPC))
    T = max(len(p) for p in planned)
    T += (-T) % GRP  # grouped tiling needs T % GRP == 0

    in_maps = []
    tiles_all = []
    for k in range(NCORES):
        m, tiles, _ = prep_core(k, h_src, c_src, embed_dst, src_f, dst_f,
                                etype, dst, T=T)
        m.update(weights)
        in_maps.append(m)
        tiles_all.append(tiles)

    if T not in _graph_cache:
        _graph_cache[T] = build_graph(T)
    nc = _graph_cache[T]

    if TRACE:
        _install_axon_hook()
    res = run_bass_kernel_spmd(nc, in_maps, list(range(NCORES)), trace=TRACE)
    LAST["res"] = res

    out = np.empty((N, 2 * H), dtype=np.float32)
    for k in range(NCORES):
        outT = np.asarray(res.results[k]["outT"]).astype(np.float32)
        for t, (n0, n1, _, _) in enumerate(tiles_all[k]):
            nn = n1 - n0
            base = k * NPC
            ng = GRP * 128
            gbase = (t // GRP) * 2 * ng + (t % GRP) * 128
            out[base + n0:base + n1, 0:H] = outT[:, gbase:gbase + nn].T
            out[base + n0:base + n1, H:2 * H] = \
                outT[:, gbase + ng:gbase + ng + nn].T
    return out
